# revision 15
# baseline (speedup 1.0000x reference)
"""Conv3D (stride (1,2,2), pad (2,3,3)) as a Bass/Tile kernel for 8 trn2 cores.

Problem: x (8,3,16,112,112) f32, weight (64,3,5,7,7), bias (64,)
      -> out (8,64,16,56,56).  Data-parallel: one batch sample per core.

Device strategy (per core, per output depth od):
  The contraction over (c=3, kw=7, kd=5) = 105 terms is packed on the PE
  partition axis and the remaining kernel dim kh=7 is a PSUM accumulation
  loop.  The host pre-builds a duplicated bf16 layout

     R[od, p=(c,kw,kd), hp, j] = Xpad[c, od+kd, hp, 2*j+kw]

  and the kernel streams bf16 matmuls (full PE rate, 1 col/cycle)
     psum_t[o, oh, j] (+)= sum_p W[p, kh, o] * R[od, p, 2*(8t+oh)+kh, j]
  with kh as the OUTER loop over the 7 row-chunks t, so the stationary
  weights only change 7 times per od (LDWEIGHTS dedup) and 7 PSUM banks
  accumulate in flight.  Bias is folded into the PSUM->SBUF eviction
  (scalar activation bias / vector tensor_scalar add), the output is
  written back as bf16 on the sync-engine HWDGE queue so the gpsimd
  SWDGE queue only carries the R loads (split 4-ways per od to engage
  more of the 16 SDMA engines).
"""

import numpy as np
import ml_dtypes

import concourse.bass as bass
import concourse.mybir as mybir
import concourse.tile as tile
from concourse import bacc
from concourse.bass_utils import run_bass_kernel_spmd

N, C, D, H, W = 8, 3, 16, 112, 112
O, KD, KH, KW = 64, 5, 7, 7
PD, PH, PW = 2, 3, 3
OD, OH, OW = 16, 56, 56
KP = C * KW * KD          # 105 contraction rows
HP = H + 2 * PH           # 118 padded input rows
OHB = 8                   # output rows per psum tile
OHC = OH // OHB           # 7 row-chunks per od
NSPLIT = 4                # s-load dma_start splits per od

BF16 = ml_dtypes.bfloat16

_CACHE = {}
LAST_RUN = None


def _build_bass():
    nc = bacc.Bacc("TRN2", target_bir_lowering=False, debug=False, num_devices=N)
    f32 = mybir.dt.float32
    bf16 = mybir.dt.bfloat16
    r = nc.dram_tensor("r", [OD, KP, HP, OW], bf16, kind="ExternalInput")
    w = nc.dram_tensor("w", [KP, KH, O], bf16, kind="ExternalInput")
    b = nc.dram_tensor("b", [O, 1], f32, kind="ExternalInput")
    out = nc.dram_tensor("out", [O, OD, OHC, OHB, OW], bf16, kind="ExternalOutput")

    with tile.TileContext(nc) as tc:
        with (
            tc.tile_pool(name="wp", bufs=1) as wp,
            tc.tile_pool(name="sp", bufs=8) as sp,
            tc.tile_pool(name="op", bufs=3) as op,
            tc.tile_pool(name="pp", bufs=8, space=bass.MemorySpace.PSUM) as pp,
        ):
            wt = wp.tile([KP, KH, O], bf16)
            nc.sync.dma_start(wt[:], w[:])
            bt = wp.tile([O, 1], f32)
            nc.sync.dma_start(bt[:], b[:])
            # PE warm-up: K=1 junk matmuls keep the HAM activity window busy
            # while the first R tile loads, so real matmuls start at 2.4 GHz.
            jt = wp.tile([1, 448], bf16)
            nc.vector.memset(jt[:], 0.0)
            psj = pp.tile([O, OHB, OW], f32, name="ps")
            for _ in range(64):
                nc.tensor.matmul(
                    psj[0:O, 0:2, 0:OW], jt[0:1, 0:64], jt[0:1, 0:112],
                    start=True, stop=True,
                )
            for od in range(OD):
                s = sp.tile([KP, HP, OW], bf16)
                if od == 0:
                    # Row-banded first load: rows 0..53 cover row-chunks
                    # t=0..2, so matmuls start before the full tile lands.
                    for h0, h1 in ((0, 54), (54, HP)):
                        for q in range(NSPLIT):
                            p0 = KP * q // NSPLIT
                            p1 = KP * (q + 1) // NSPLIT
                            nc.gpsimd.dma_start(
                                s[p0:p1, h0:h1], r[od, p0:p1, h0:h1]
                            )
                else:
                    for q in range(NSPLIT):
                        p0 = KP * q // NSPLIT
                        p1 = KP * (q + 1) // NSPLIT
                        nc.gpsimd.dma_start(s[p0:p1], r[od, p0:p1])
                ob = op.tile([O, OHC, OHB, OW], bf16)
                pss = [pp.tile([O, OHB, OW], f32, name="ps") for _ in range(OHC)]
                for kh in range(KH):
                    for t in range(OHC):
                        base = 2 * OHB * t + kh
                        nc.tensor.matmul(
                            pss[t][:], wt[0:KP, kh, :],
                            s[0:KP, base : base + 2 * OHB : 2, :],
                            start=(kh == 0), stop=(kh == KH - 1),
                        )
                last = od == OD - 1
                for t in range(OHC):
                    if last and t == OHC - 1:
                        # Final tile: drain via both engines at once so the
                        # closing out-DMA unblocks as early as possible.
                        nc.scalar.activation(
                            ob[0:O, t, 0:4], pss[t][0:O, 0:4],
                            mybir.ActivationFunctionType.Identity,
                            bias=bt[0:O, 0:1],
                        )
                        nc.vector.tensor_scalar_add(
                            ob[0:O, t, 4:OHB], pss[t][0:O, 4:OHB], bt[0:O, 0:1]
                        )
                    elif t % 2 == 0:
                        nc.scalar.activation(
                            ob[0:O, t], pss[t][:],
                            mybir.ActivationFunctionType.Identity,
                            bias=bt[0:O, 0:1],
                        )
                    else:
                        nc.vector.tensor_scalar_add(ob[0:O, t], pss[t][:], bt[0:O, 0:1])
                    if last and t == 3:
                        nc.sync.dma_start(out[0:O, od, 0:4], ob[0:O, 0:4])
                    if last and t == 5:
                        nc.sync.dma_start(out[0:O, od, 4:6], ob[0:O, 4:6])
                if last:
                    nc.sync.dma_start(out[0:O, od, 6:OHC], ob[0:O, 6:OHC])
                else:
                    nc.sync.dma_start(out[0:O, od], ob[:])
    nc.compile()
    return nc


def _host_pack(x, weight, bias):
    """Build the pre-shifted bf16 rhs volume R per sample plus weight/bias tiles."""
    xf = np.ascontiguousarray(x, dtype=np.float32)
    xp = np.zeros((N, C, D + 2 * PD, HP, W + 2 * PW), np.float32)
    xp[:, :, PD : PD + D, PH : PH + H, PW : PW + W] = xf
    xpb = xp.astype(BF16)

    # R[n, od, p=(c,kw,kd), hp, j] = xpb[n, c, od+kd, hp, 2*j+kw]
    sn, sc, sd, sh, sw = xpb.strides
    Rv = np.lib.stride_tricks.as_strided(
        xpb,
        shape=(N, OD, C, KW, KD, HP, OW),
        strides=(sn, sd, sc, sw, sd, sh, 2 * sw),
    )
    R = np.ascontiguousarray(Rv).reshape(N, OD, KP, HP, OW)

    # Wt[p=(c,kw,kd), kh, o]
    Wt = (
        np.asarray(weight, np.float32)
        .transpose(1, 4, 2, 3, 0)  # [C, KW, KD, KH, O]
        .reshape(KP, KH, O)
        .astype(BF16)
    )
    b2 = np.asarray(bias, np.float32).reshape(O, 1)
    return R, Wt, b2


def kernel(x, weight, bias):
    global LAST_RUN
    if "nc" not in _CACHE:
        _CACHE["nc"] = _build_bass()
    nc = _CACHE["nc"]

    R, Wt, b2 = _host_pack(x, weight, bias)
    in_maps = [{"r": R[n], "w": Wt, "b": b2} for n in range(N)]
    res = run_bass_kernel_spmd(nc, in_maps, core_ids=list(range(N)))
    LAST_RUN = res
    out = np.stack(
        [res.results[n]["out"].reshape(O, OD, OH, OW) for n in range(N)], axis=0
    )
    return out.astype(np.float32)
